# revision 1
# baseline (speedup 1.0000x reference)
"""RNN-T Joiner kernel for 8 Trainium2 NeuronCores.

Reference computation (per batch element n):
    enc = encoder_out[n] @ W_enc.T + b_enc          # (T=200, J=512)
    dec = decoder_out[n] @ W_dec.T + b_dec          # (U=50,  J=512)
    x   = tanh(enc[:,None,:] + dec[None,:,:])       # (T, U, J)
    out = x @ W_out.T + b_out                       # (T, U, V=500)

Sharding: data-parallel over N=8 (one batch element per core).

Device-side dataflow (everything j/c-major, pre-transposed on host):
    PE:     encT/decT projections (fp32), main matmul (bf16, fp32 PSUM)
    GPSIMD: S[j,t,u] = encT[j,t] + decT[j,u]  (broadcast add, bf16 out)
    ACT:    XT = tanh(S)  (bf16)
    DVE:    PSUM -> SBUF evacuation + b_out bias (batched 4 M-tiles/instr)
    DMA:    1MB contiguous output stores
"""

import numpy as np

N, T, U = 8, 200, 50
C = 512   # enc/dec feature dim
J = 512   # joint dim
V = 500   # vocab
TU = T * U
P = 128
KC = J // P          # 4 contraction chunks of 128
T_BLK = 64           # t's per block (64*50 = 3200 tu = 25 M-tiles of 128)
BLK_TU = T_BLK * U   # 3200
GROUP = 4            # M-tiles per PSUM tile / output DMA group

_CACHE = {}


def _build_bass():
    import concourse.bass as bass  # noqa: F401
    import concourse.mybir as mybir
    import concourse.tile as tile
    from concourse import bacc

    bf16 = mybir.dt.bfloat16
    f32 = mybir.dt.float32
    Act = mybir.ActivationFunctionType

    nc = bacc.Bacc("TRN2", target_bir_lowering=False, debug=False, num_devices=N)

    enc_in = nc.dram_tensor("enc_in", [C, T], bf16, kind="ExternalInput").ap()
    dec_in = nc.dram_tensor("dec_in", [C, U], bf16, kind="ExternalInput").ap()
    w_enc = nc.dram_tensor("w_enc", [C, J], bf16, kind="ExternalInput").ap()
    w_dec = nc.dram_tensor("w_dec", [C, J], bf16, kind="ExternalInput").ap()
    w_out = nc.dram_tensor("w_out", [J, V], bf16, kind="ExternalInput").ap()
    biases = nc.dram_tensor("biases", [P, 2 * KC + V], f32,
                            kind="ExternalInput").ap()
    logits = nc.dram_tensor("logits", [TU, V], bf16, kind="ExternalOutput").ap()

    n_full = TU // P          # 78 full M-tiles
    tail = TU - n_full * P    # 16

    with tile.TileContext(nc) as tc:
        with (
            tc.tile_pool(name="const", bufs=1) as const,
            tc.tile_pool(name="s", bufs=8) as sp,
            tc.tile_pool(name="xt", bufs=12) as xtp,
            tc.tile_pool(name="lout", bufs=3) as lp,
            tc.tile_pool(name="ps", bufs=2, space="PSUM") as psp,
        ):
            # ---- load weights + inputs -------------------------------------
            w_enc_sb = const.tile([P, KC, J], bf16)
            w_dec_sb = const.tile([P, KC, J], bf16)
            w_out_sb = const.tile([P, KC, V], bf16)
            enc_in_sb = const.tile([P, KC, T], bf16)
            dec_in_sb = const.tile([P, KC, U], bf16)
            enc_in_r = enc_in.rearrange("(kc p) t -> p kc t", p=P)
            dec_in_r = dec_in.rearrange("(kc p) u -> p kc u", p=P)
            w_enc_r = w_enc.rearrange("(kc p) j -> p kc j", p=P)
            w_dec_r = w_dec.rearrange("(kc p) j -> p kc j", p=P)
            bias_sb = const.tile([P, 2 * KC + V], f32)
            nc.sync.dma_start(bias_sb[:], biases)
            b_enc_sb = bias_sb[:, 0:KC]
            b_dec_sb = bias_sb[:, KC:2 * KC]
            b_out_sb = bias_sb[:, 2 * KC:]
            nc.sync.dma_start(dec_in_sb[:], dec_in_r)
            nc.scalar.dma_start(enc_in_sb[:], enc_in_r)
            nc.sync.dma_start(w_dec_sb[:], w_dec_r)
            nc.scalar.dma_start(w_enc_sb[:], w_enc_r)
            nc.scalar.dma_start(
                w_out_sb[:], w_out.rearrange("(kc p) v -> p kc v", p=P))

            # ---- input projections, directly in transposed (j-major) form --
            encT = const.tile([P, KC, T], f32)
            decT = const.tile([P, KC, U], f32)
            for jc in range(KC):
                ps = psp.tile([P, GROUP, 512], f32, tag="ps", name="pse")
                for kc in range(KC):
                    nc.tensor.matmul(
                        ps[:, 0, :T],
                        lhsT=w_enc_sb[:, kc, jc * P:(jc + 1) * P],
                        rhs=enc_in_sb[:, kc, :],
                        start=(kc == 0),
                        stop=(kc == KC - 1),
                    )
                nc.scalar.activation(
                    encT[:, jc, :], ps[:, 0, :T], Act.Identity,
                    bias=b_enc_sb[:, jc:jc + 1],
                )
                ps = psp.tile([P, GROUP, 512], f32, tag="ps", name="psd")
                for kc in range(KC):
                    nc.tensor.matmul(
                        ps[:, 0, :U],
                        lhsT=w_dec_sb[:, kc, jc * P:(jc + 1) * P],
                        rhs=dec_in_sb[:, kc, :],
                        start=(kc == 0),
                        stop=(kc == KC - 1),
                    )
                nc.scalar.activation(
                    decT[:, jc, :], ps[:, 0, :U], Act.Identity,
                    bias=b_dec_sb[:, jc:jc + 1],
                )

            # ---- XT block production (GPSIMD add -> ACT tanh) --------------
            # xts[block][kc] -> flattened [P, block_tu] bf16 AP
            xts = []
            t0 = 0
            blk = 0
            while t0 < T:
                nt = min(T_BLK, T - t0)
                row = []
                # sub-chunk the first block so matmuls can start early
                nsub = 4 if blk == 0 else 1
                step = nt // nsub
                kc_tiles = []
                for kc in range(KC):
                    s_full = sp.tile([P, T_BLK, U], bf16, tag="s", name=f"s{kc}")
                    x_full = xtp.tile([P, T_BLK, U], bf16, tag="xt", name=f"xt{kc}")
                    kc_tiles.append((s_full[:, :nt, :], x_full[:, :nt, :]))
                for sub in range(nsub):
                    lo, hi = sub * step, (sub + 1) * step
                    for kc in range(KC):
                        s, x = kc_tiles[kc]
                        eng = (nc.vector
                               if (blk < 2 and kc == 0) or (blk == 0 and kc == 2)
                               else nc.gpsimd)
                        eng.tensor_add(
                            s[:, lo:hi, :],
                            encT[:, kc, t0 + lo:t0 + hi, None]
                            .to_broadcast((P, hi - lo, U)),
                            decT[:, kc, None, :].to_broadcast((P, hi - lo, U)),
                        )
                        nc.scalar.activation(
                            x[:, lo:hi, :], s[:, lo:hi, :], Act.Tanh
                        )
                for kc in range(KC):
                    row.append(kc_tiles[kc][1].rearrange("p t u -> p (t u)"))
                xts.append(row)
                t0 += nt
                blk += 1

            # ---- main matmul over flat M-tile groups -----------------------
            def lhsT_for(kc, m_lo, rows):
                b = m_lo // BLK_TU
                off = m_lo - b * BLK_TU
                return xts[b][kc][:, off:off + rows]

            m = 0
            while m < n_full:
                g = min(GROUP, n_full - m)
                take_tail = (m + g == n_full) and tail and (g < GROUP)
                nsub = g + (1 if take_tail else 0)
                ps = psp.tile([P, GROUP, 512], f32, tag="ps", name="psm")
                L = lp.tile([P, GROUP, V], bf16, tag="L", name="L")
                for i in range(nsub):
                    rows = P if i < g else tail
                    for kc in range(KC):
                        nc.tensor.matmul(
                            ps[:rows, i, :V],
                            lhsT=lhsT_for(kc, (m + i) * P, rows),
                            rhs=w_out_sb[:, kc, :],
                            start=(kc == 0),
                            stop=(kc == KC - 1),
                        )
                nc.vector.tensor_add(
                    L[:, :nsub, :],
                    ps[:, :nsub, :V],
                    b_out_sb[:, None, :].to_broadcast((P, nsub, V)),
                )
                nc.sync.dma_start(
                    logits[m * P:(m + g) * P, :].rearrange("(i p) v -> p i v", p=P),
                    L[:, :g, :],
                )
                if take_tail:
                    nc.sync.dma_start(
                        logits[n_full * P:TU, :],
                        L[:tail, g, :],
                    )
                m += g

    nc.compile()
    return nc


def _get_bass():
    if "nc" not in _CACHE:
        _CACHE["nc"] = _build_bass()
    return _CACHE["nc"]


def _pack_inputs(inputs):
    import ml_dtypes

    encoder_out = np.ascontiguousarray(
        np.asarray(inputs["encoder_out"], np.float32).transpose(0, 2, 1)
        .astype(ml_dtypes.bfloat16))
    decoder_out = np.ascontiguousarray(
        np.asarray(inputs["decoder_out"], np.float32).transpose(0, 2, 1)
        .astype(ml_dtypes.bfloat16))
    WencT = np.ascontiguousarray(
        np.asarray(inputs["W_enc"], np.float32).T.astype(ml_dtypes.bfloat16))
    WdecT = np.ascontiguousarray(
        np.asarray(inputs["W_dec"], np.float32).T.astype(ml_dtypes.bfloat16))
    WoutT = np.ascontiguousarray(
        np.asarray(inputs["W_out"], np.float32).T.astype(ml_dtypes.bfloat16))
    biases = np.empty((P, 2 * KC + V), np.float32)
    biases[:, 0:KC] = np.asarray(inputs["b_enc"], np.float32).reshape(KC, P).T
    biases[:, KC:2 * KC] = (
        np.asarray(inputs["b_dec"], np.float32).reshape(KC, P).T)
    biases[:, 2 * KC:] = np.asarray(inputs["b_out"], np.float32)[None, :]
    return [
        {
            "enc_in": encoder_out[n],
            "dec_in": decoder_out[n],
            "w_enc": WencT,
            "w_dec": WdecT,
            "w_out": WoutT,
            "biases": biases,
        }
        for n in range(N)
    ]


def run(inputs, trace=False):
    """Run the bass kernel; returns (output array, BassKernelResults)."""
    from concourse.bass_utils import run_bass_kernel_spmd

    nc = _get_bass()
    in_maps = _pack_inputs(inputs)
    res = run_bass_kernel_spmd(nc, in_maps, core_ids=list(range(N)), trace=trace)
    out = np.stack([np.asarray(r["logits"], dtype=np.float32)
                    for r in res.results])
    return out.reshape(N, T, U, V), res


def kernel(**inputs):
    out, _ = run(inputs)
    return out



# revision 2
# speedup vs baseline: 1.1668x; 1.1668x over previous
"""RNN-T Joiner kernel for 8 Trainium2 NeuronCores.

Reference computation (per batch element n):
    enc = encoder_out[n] @ W_enc.T + b_enc          # (T=200, J=512)
    dec = decoder_out[n] @ W_dec.T + b_dec          # (U=50,  J=512)
    x   = tanh(enc[:,None,:] + dec[None,:,:])       # (T, U, J)
    out = x @ W_out.T + b_out                       # (T, U, V=500)

Sharding: data-parallel over N=8 (one batch element per core).

Device-side dataflow (per core):
    PE:     warmup MMs (HAM un-throttle), enc/dec projections, then the
            main matmul with W_out chunks stationary (V padded to 512 ->
            4 vtiles of 128, FWL-eligible) and x streaming as the moving
            operand.  Output is logitsT [VP=512, TU=10000] (v on
            partitions); the host un-transposes with cheap numpy views.
    DVE:    broadcast adds S = encT + decT in bf16 (2x packed mode: the
            dec side is pre-replicated into decRep so both operands have
            unit inner stride), plus most PSUM evacuations
            (tensor_scalar_add with per-partition b_out).
    ACT:    tanh (bf16), projection bias-adds, 1/4 of the evacuations.
    GPSIMD: one-time decRep replication builds only.
    DMA:    one big store per t-block (up to 2 MB contiguous rows).
"""

import numpy as np

N, T, U = 8, 200, 50
C = 512   # enc/dec feature dim
J = 512   # joint dim
V = 500   # vocab
VP = 512  # padded vocab (4 vtiles of 128)
TU = T * U
P = 128
KC = J // P              # 4 contraction chunks of 128
TBS = [10, 20, 40, 40, 40, 40, 10]   # t-block sizes (sum = 200)
T_B = max(TBS)           # 40
SUB = 500                # MM moving free-dim per sub-block
NV = VP // P             # 4 vtiles
WARMUP = 16              # junk MMs to warm the PE / HAM

_CACHE = {}


def _build_bass():
    import concourse.bass as bass  # noqa: F401
    import concourse.mybir as mybir
    import concourse.tile as tile
    from concourse import bacc

    bf16 = mybir.dt.bfloat16
    f32 = mybir.dt.float32
    Act = mybir.ActivationFunctionType

    nc = bacc.Bacc("TRN2", target_bir_lowering=False, debug=False, num_devices=N)

    enc_in = nc.dram_tensor("enc_in", [C, T], bf16, kind="ExternalInput").ap()
    dec_in = nc.dram_tensor("dec_in", [C, U], bf16, kind="ExternalInput").ap()
    w_enc = nc.dram_tensor("w_enc", [C, J], bf16, kind="ExternalInput").ap()
    w_dec = nc.dram_tensor("w_dec", [C, J], bf16, kind="ExternalInput").ap()
    w_out = nc.dram_tensor("w_out", [J, VP], bf16, kind="ExternalInput").ap()
    biases = nc.dram_tensor("biases", [P, 3 * KC], f32,
                            kind="ExternalInput").ap()
    logits = nc.dram_tensor("logits", [VP, TU], bf16, kind="ExternalOutput").ap()

    with tile.TileContext(nc) as tc:
        with (
            tc.tile_pool(name="const", bufs=1) as const,
            tc.tile_pool(name="s", bufs=2) as sp,
            tc.tile_pool(name="xt", bufs=3) as xtp,
            tc.tile_pool(name="lout", bufs=2) as lp,
            tc.tile_pool(name="ps", bufs=2, space="PSUM") as psp,
        ):
            # ---- constants / inputs ----------------------------------------
            w_enc_sb = const.tile([P, KC, J], bf16)
            w_dec_sb = const.tile([P, KC, J], bf16)
            w_out_sb = const.tile([P, KC, VP], bf16)
            enc_in_sb = const.tile([P, KC, T], bf16)
            dec_in_sb = const.tile([P, KC, U], bf16)
            bias_sb = const.tile([P, 3 * KC], f32)
            b_enc_sb = bias_sb[:, 0:KC]
            b_dec_sb = bias_sb[:, KC:2 * KC]
            b_out_sb = bias_sb[:, 2 * KC:3 * KC]

            encT = const.tile([P, KC, T], bf16)
            decT = const.tile([P, KC, U], bf16)
            decRep = const.tile([P, KC, U, T_B], bf16)
            junk = const.tile([P, 512], bf16)

            # loads: dec path + w_out on sync, bias/enc path on scalar
            nc.scalar.dma_start(bias_sb[:], biases)
            nc.sync.dma_start(dec_in_sb[:],
                              dec_in.rearrange("(kc p) u -> p kc u", p=P))
            nc.sync.dma_start(w_dec_sb[:],
                              w_dec.rearrange("(kc p) j -> p kc j", p=P))
            nc.scalar.dma_start(enc_in_sb[:],
                                enc_in.rearrange("(kc p) t -> p kc t", p=P))
            nc.scalar.dma_start(w_enc_sb[:],
                                w_enc.rearrange("(kc p) j -> p kc j", p=P))
            nc.sync.dma_start(w_out_sb[:],
                              w_out.rearrange("(kc p) v -> p kc v", p=P))

            nc.vector.memset(junk[:], 0.0)

            # ---- dec projection (tiny, first) ------------------------------
            for jc in range(KC):
                ps = psp.tile([P, NV, 512], f32, tag="ps", name="psd")
                for kc in range(KC):
                    nc.tensor.matmul(
                        ps[:, 0, :U],
                        lhsT=w_dec_sb[:, kc, jc * P:(jc + 1) * P],
                        rhs=dec_in_sb[:, kc, :],
                        start=(kc == 0),
                        stop=(kc == KC - 1),
                    )
                nc.scalar.activation(
                    decT[:, jc, :], ps[:, 0, :U], Act.Identity,
                    bias=b_dec_sb[:, jc:jc + 1],
                )

            # ---- enc projection --------------------------------------------
            for jc in range(KC):
                ps = psp.tile([P, NV, 512], f32, tag="ps", name="pse")
                for kc in range(KC):
                    nc.tensor.matmul(
                        ps[:, 0, :T],
                        lhsT=w_enc_sb[:, kc, jc * P:(jc + 1) * P],
                        rhs=enc_in_sb[:, kc, :],
                        start=(kc == 0),
                        stop=(kc == KC - 1),
                    )
                nc.scalar.activation(
                    encT[:, jc, :], ps[:, 0, :T], Act.Identity,
                    bias=b_enc_sb[:, jc:jc + 1],
                )

            # ---- PE warmup: junk matmuls to keep HAM un-throttled ----------
            wps = psp.tile([P, NV, 512], f32, tag="ps", name="psw")
            for i in range(WARMUP):
                nc.tensor.matmul(
                    wps[:, i % NV, :],
                    lhsT=junk[:, :P],
                    rhs=junk[:],
                    start=True,
                    stop=True,
                )

            # ---- decRep: replicate decT along t (for 2x-mode DVE adds) -----
            # kc 0/1 on gpsimd, kc 2/3 on vector (parallel build)
            for kc in range(KC):
                eng = nc.gpsimd if kc < 2 else nc.vector
                eng.tensor_copy(
                    decRep[:, kc],
                    decT[:, kc, :, None].to_broadcast((P, U, T_B)),
                )

            # ---- main pipeline over t-blocks -------------------------------
            t0 = 0
            tu0 = 0
            for b, tbs in enumerate(TBS):
                ntu = U * tbs
                nsub = ntu // SUB
                S = sp.tile([P, KC, U * T_B], bf16, tag="s", name="S")
                X = xtp.tile([P, KC, U * T_B], bf16, tag="xt", name="X")
                for kc in range(KC):
                    s3 = S[:, kc, :ntu].rearrange("p (u t) -> p u t", t=tbs)
                    enc_b = (encT[:, kc, None, t0:t0 + tbs]
                             .to_broadcast((P, U, tbs)))
                    if b == 0:
                        # decRep not built yet: direct strided broadcast
                        dec_b = (decT[:, kc, :, None]
                                 .to_broadcast((P, U, tbs)))
                    else:
                        dec_b = decRep[:, kc, :, :tbs]
                    nc.vector.tensor_add(s3, enc_b, dec_b)
                    nc.scalar.activation(
                        X[:, kc, :ntu], S[:, kc, :ntu], Act.Tanh)

                L = lp.tile([P, NV, U * T_B], bf16, tag="L", name="L")
                for v in range(NV):
                    ps = psp.tile([P, NV, 512], f32, tag="ps", name="psm")
                    for kc in range(KC):
                        for s in range(nsub):
                            nc.tensor.matmul(
                                ps[:, s, :SUB],
                                lhsT=w_out_sb[:, kc, v * P:(v + 1) * P],
                                rhs=X[:, kc, s * SUB:(s + 1) * SUB],
                                start=(kc == 0),
                                stop=(kc == KC - 1),
                            )
                    lv = L[:, v, :ntu].rearrange("p (s c) -> p s c", c=SUB)
                    if v == NV - 1:
                        nc.scalar.activation(
                            lv, ps[:, :nsub, :SUB], Act.Identity,
                            bias=b_out_sb[:, v:v + 1],
                        )
                    else:
                        nc.vector.tensor_scalar_add(
                            lv, ps[:, :nsub, :SUB], b_out_sb[:, v:v + 1])
                nc.sync.dma_start(
                    logits[:, tu0:tu0 + ntu]
                    .rearrange("(v p) c -> p v c", p=P),
                    L[:, :, :ntu],
                )
                t0 += tbs
                tu0 += ntu

    nc.compile()
    return nc


def _get_bass():
    if "nc" not in _CACHE:
        _CACHE["nc"] = _build_bass()
    return _CACHE["nc"]


def _pack_inputs(inputs):
    import ml_dtypes

    encoder_out = np.ascontiguousarray(
        np.asarray(inputs["encoder_out"], np.float32).transpose(0, 2, 1)
        .astype(ml_dtypes.bfloat16))
    decoder_out = np.ascontiguousarray(
        np.asarray(inputs["decoder_out"], np.float32).transpose(0, 2, 1)
        .astype(ml_dtypes.bfloat16))
    WencT = np.ascontiguousarray(
        np.asarray(inputs["W_enc"], np.float32).T.astype(ml_dtypes.bfloat16))
    WdecT = np.ascontiguousarray(
        np.asarray(inputs["W_dec"], np.float32).T.astype(ml_dtypes.bfloat16))
    Wout_pad = np.zeros((VP, J), np.float32)
    Wout_pad[:V] = np.asarray(inputs["W_out"], np.float32)
    WoutT = np.ascontiguousarray(Wout_pad.T.astype(ml_dtypes.bfloat16))
    b_out_pad = np.zeros((VP,), np.float32)
    b_out_pad[:V] = np.asarray(inputs["b_out"], np.float32)
    biases = np.empty((P, 3 * KC), np.float32)
    biases[:, 0:KC] = np.asarray(inputs["b_enc"], np.float32).reshape(KC, P).T
    biases[:, KC:2 * KC] = (
        np.asarray(inputs["b_dec"], np.float32).reshape(KC, P).T)
    biases[:, 2 * KC:3 * KC] = b_out_pad.reshape(KC, P).T
    return [
        {
            "enc_in": encoder_out[n],
            "dec_in": decoder_out[n],
            "w_enc": WencT,
            "w_dec": WdecT,
            "w_out": WoutT,
            "biases": biases,
        }
        for n in range(N)
    ]


def _unpack_output(res):
    """logitsT [VP, TU] (block-major, u-major-within-block) -> (T, U, V)."""
    out = np.empty((N, T, U, V), np.float32)
    for n, r in enumerate(res.results):
        arr = np.asarray(r["logits"], dtype=np.float32)   # [VP, TU]
        o = 0
        t0 = 0
        for tbs in TBS:
            seg = arr[:, o:o + U * tbs].reshape(VP, U, tbs)
            out[n, t0:t0 + tbs] = seg.transpose(2, 1, 0)[:, :, :V]
            o += U * tbs
            t0 += tbs
    return out


def run(inputs, trace=False):
    """Run the bass kernel; returns (output array, BassKernelResults)."""
    from concourse.bass_utils import run_bass_kernel_spmd

    nc = _get_bass()
    in_maps = _pack_inputs(inputs)
    res = run_bass_kernel_spmd(nc, in_maps, core_ids=list(range(N)), trace=trace)
    return _unpack_output(res), res


def kernel(**inputs):
    out, _ = run(inputs)
    return out
